# revision 39
# baseline (speedup 1.0000x reference)
"""Bass/Trainium2 kernel for BiasedMultiheadAttention.

Problem shapes (hardcoded): B=2, L=2048, D=1024, H=16, d=64.
Sharding: 8 cores = 2 batches x 4 head-groups (4 heads per core).
Each core computes its heads' attention and a partial out-projection;
host sums the 4 partials per batch and adds b_out.

Key compaction: key_padding_mask ~ Bernoulli(0.5) marks ~half the keys
padded, and padded keys contribute exactly 0 attention mass (the
reference's -1e4 shift -> clamped exp(-20) ~ 2e-9, negligible). The host
gathers the unpadded keys per batch and the kernel runs attention over
Lk ~ 1100 keys instead of 2048, halving QK/AV/exp/bias traffic. The
compiled kernel is cached per rounded Lk.

Device dataflow per core (b, heads h0..h0+3):
  in-proj  : qT [d,Lq] from x (q weights pre-scaled 1/sqrt(d)),
             kT [d,Lk] and v [Lk,d] from the gathered key rows
  bias     : host-precomputed exp(attn_bias)^T over kept keys, bf16;
             applied POST-exp as a DVE bf16 multiply (2x mode) so the PE
             never touches bias (no identity-preload matmuls)
  QK^T     : S^T[k,q] into a 2-bank [128,1024] psum pair per key-chunk,
             2-head row-tiled (K=64 at array rows 0-63 / 64-127)
  softmax  : one ACT exp per pair; prob = exp(s)*exp(bias), identical to
             softmax(s+bias-1e4*pad) since the max-shift cancels
  AV + Z   : aug-v (ones column) M=65 matmuls accumulate O^T rows 0-63 and
             the softmax denominator Z in row 64 of each head's bank
  norm     : DVE evacuation of O^T/Z, reciprocal(Z), K=1 ones-matmul
             broadcast, DVE multiply; head 2hp+1's normalized O^T hops to
             SBUF partitions 64-127 via an SBUF->SBUF DMA so the
             out-projection can contract K=128 (2 heads per matmul)
  out-proj : partial[l,j] over this core's 256 head-dims, K=128 x2, fp32
"""

import numpy as np
import ml_dtypes

B, L, D, H = 2, 2048, 1024, 16
NHC = 4          # heads per core
d = 64
QB = 512         # query block (matmul moving free dim)

_BF16 = ml_dtypes.bfloat16

_cached = {}
_PACK_OPROJ = True


def _build_nc(Lx=L, loop_n=1, Lk=1152, pack_oproj=True):
    import contextlib

    import concourse.bacc as bacc
    import concourse.mybir as mybir
    import concourse.tile as tile

    fp32 = mybir.dt.float32
    fp16 = mybir.dt.float16
    bf16 = mybir.dt.bfloat16
    Exp = mybir.ActivationFunctionType.Exp

    nqb = Lx // QB
    nkc = Lk // 128
    nlc = Lx // 128

    # moving-dim chunks for the k in-projection (<=512 each)
    kchunks = []
    off = 0
    while off < Lk:
        w = min(QB, Lk - off)
        kchunks.append((off, w))
        off += w

    nc = bacc.Bacc("TRN2", target_bir_lowering=False)

    xT_d = nc.dram_tensor("xT", [D, Lx], bf16, kind="ExternalInput")
    xkT_d = nc.dram_tensor("xkT", [D, Lk], bf16, kind="ExternalInput")
    wqkT_d = nc.dram_tensor("wqkT", [D, 512], bf16, kind="ExternalInput")
    wvT_d = nc.dram_tensor("wvT", [D, 256], bf16, kind="ExternalInput")
    woT_d = nc.dram_tensor("woT", [128, 2, D], bf16, kind="ExternalInput")
    woT4_d = nc.dram_tensor("woT4", [64, 4, D], bf16, kind="ExternalInput")
    bqk_d = nc.dram_tensor("bqk", [128, 4], fp32, kind="ExternalInput")
    bvr_d = nc.dram_tensor("bvr", [128, 256], fp32, kind="ExternalInput")
    biasT_d = nc.dram_tensor(
        "biasT", [2, nkc, nqb, 128, 2 * QB], bf16, kind="ExternalInput"
    )
    out_d = nc.dram_tensor("partial", [Lx, D], fp16, kind="ExternalOutput")
    # DRAM staging for the head-B partition hop (SBUF->SBUF DMA hangs the HW)
    stg_d = nc.dram_tensor("stg_hop", [2, Lx // QB, 64, QB], bf16, kind="Internal")

    with tile.TileContext(nc) as tc:
        with contextlib.ExitStack() as ctx:
            const = ctx.enter_context(tc.tile_pool(name="const", bufs=1))
            iobuf = ctx.enter_context(tc.tile_pool(name="iobuf", bufs=2))
            biasp = ctx.enter_context(tc.tile_pool(name="biasp", bufs=18))
            probsp = ctx.enter_context(tc.tile_pool(name="probsp", bufs=6))
            outp = ctx.enter_context(tc.tile_pool(name="outp", bufs=3))
            zrecp = ctx.enter_context(tc.tile_pool(name="zrecp", bufs=2))
            stagep = ctx.enter_context(tc.tile_pool(name="stagep", bufs=2))
            # 8 PSUM banks: 2x2 (big shared slots) + 2x1 (ot) + 2x1 (outproj)
            ps_qk = ctx.enter_context(tc.tile_pool(name="ps_qk", bufs=2, space="PSUM"))
            ps_ot = ctx.enter_context(tc.tile_pool(name="ps_ot", bufs=2, space="PSUM"))
            ps_out = ctx.enter_context(tc.tile_pool(name="ps_out", bufs=2, space="PSUM"))
            ps_misc = ps_qk

            def _emit():
                # ---- persistent SBUF ----
                xT_sb = const.tile([128, 8, Lx], bf16, name="xT_sb", tag="xT_sb")
                xT_r = xT_d.rearrange("(dc p) l -> p dc l", p=128)
                for dc in range(8):
                    nc.sync.dma_start(xT_sb[:, dc, :], xT_r[:, dc, :])
                xkT_sb = const.tile([128, 8, Lk], bf16, name="xkT_sb", tag="xkT_sb")
                nc.sync.dma_start(
                    xkT_sb[:], xkT_d.rearrange("(dc p) l -> p dc l", p=128)
                )
                wqkT_sb = const.tile([128, 8, 512], bf16, name="wqkT_sb", tag="wqkT_sb")
                nc.sync.dma_start(
                    wqkT_sb[:], wqkT_d.rearrange("(dc p) f -> p dc f", p=128)
                )
                wvT_sb = const.tile([128, 8, 256], bf16, name="wvT_sb", tag="wvT_sb")
                nc.sync.dma_start(
                    wvT_sb[:], wvT_d.rearrange("(dc p) f -> p dc f", p=128)
                )
                if pack_oproj:
                    woT_sb2 = const.tile(
                        [128, 2, D], bf16, name="woT_sb2", tag="woT_sb2"
                    )
                    nc.sync.dma_start(woT_sb2[:], woT_d[:])
                else:
                    woT_sb2 = const.tile(
                        [64, 4, D], bf16, name="woT_sb2", tag="woT_sb2"
                    )
                    nc.sync.dma_start(woT_sb2[:], woT4_d[:])
                bqk_sb = const.tile([128, 4], fp32, name="bqk_sb", tag="bqk_sb")
                nc.sync.dma_start(bqk_sb[:], bqk_d[:])
                bvr_sb = const.tile([128, 256], fp32, name="bvr_sb", tag="bvr_sb")
                nc.sync.dma_start(bvr_sb[:], bvr_d[:])

                qT_sb = iobuf.tile([128, 2, Lx], bf16, name="qT_sb", tag="qT_sb")
                kT_sb = iobuf.tile([128, 2, Lk], bf16, name="kT_sb", tag="kT_sb")
                v_sb = iobuf.tile([128, nkc, 4, 65], bf16, name="v_sb", tag="v_sb")
                nc.vector.memset(v_sb[:, :, :, 64:65], 1.0)
                if pack_oproj:
                    otn_sb = iobuf.tile(
                        [128, 2, Lx], bf16, name="otn_sb", tag="otn_sb"
                    )
                else:
                    otn_sb = iobuf.tile(
                        [64, 4, Lx], bf16, name="otn_sb", tag="otn_sb"
                    )
                ones_f32 = const.tile([65, 64], fp32, name="ones_f32", tag="ones_f32")
                nc.vector.memset(ones_f32[:], 1.0)

                # ---- in-projection (emitted piecewise, see lead/units below)
                def _inproj_q_nb(m, nb):
                    ps = ps_misc.tile(
                        [128, 2 * QB], fp32, name="ps_iq", tag="ps_qk"
                    )[:, 0:QB]
                    for dc in range(8):
                        nc.tensor.matmul(
                            ps[:],
                            wqkT_sb[:, dc, m * 128 : (m + 1) * 128],
                            xT_sb[:, dc, nb * QB : (nb + 1) * QB],
                            start=(dc == 0),
                            stop=(dc == 7),
                        )
                    nc.vector.tensor_scalar_add(
                        qT_sb[:, m, nb * QB : (nb + 1) * QB],
                        ps[:],
                        bqk_sb[:, m : m + 1],
                    )

                def _inproj_k_chunk(m, ci):
                    off, w = kchunks[ci]
                    ps = ps_misc.tile(
                        [128, 2 * QB], fp32, name="ps_ik", tag="ps_qk"
                    )[:, 0:w]
                    for dc in range(8):
                        nc.tensor.matmul(
                            ps[:],
                            wqkT_sb[:, dc, (2 + m) * 128 : (3 + m) * 128],
                            xkT_sb[:, dc, off : off + w],
                            start=(dc == 0),
                            stop=(dc == 7),
                        )
                    nc.vector.tensor_scalar_add(
                        kT_sb[:, m, off : off + w],
                        ps[:],
                        bqk_sb[:, 2 + m : 3 + m],
                    )

                def _inproj_v_lc(lc):
                    ps = ps_misc.tile(
                        [128, 2 * QB], fp32, name="ps_iv", tag="ps_qk"
                    )
                    psv = ps[:, :256]
                    for dc in range(8):
                        nc.tensor.matmul(
                            psv,
                            xkT_sb[:, dc, lc * 128 : (lc + 1) * 128],
                            wvT_sb[:, dc, :],
                            start=(dc == 0),
                            stop=(dc == 7),
                        )
                    nc.vector.tensor_add(
                        v_sb[:, lc, :, 0:64],
                        psv.rearrange("p (h x) -> p h x", h=4),
                        bvr_sb.rearrange("p (h x) -> p h x", h=4),
                    )

                # Minimal serial lead before attention can start: q block 0,
                # all of k for head-pair 0, first two v chunks. Everything
                # else drips into the qb0 kc-loop slots (PE fills ACT's
                # latency shadow), keeping ACT's exp stream nearly gapless.
                _inproj_q_nb(0, 0)
                for ci in range(len(kchunks)):
                    _inproj_k_chunk(0, ci)
                _inproj_v_lc(0)
                _inproj_v_lc(1)
                vs = [
                    (lambda lc=lc: _inproj_v_lc(lc)) for lc in range(2, nkc)
                ]
                others = [
                    (lambda ci=ci: _inproj_k_chunk(1, ci))
                    for ci in range(len(kchunks))
                ] + [lambda: _inproj_q_nb(1, 0)]
                early_units = []
                for i in range(max(len(vs), len(others))):
                    if i < len(vs):
                        early_units.append(vs[i])
                    if i < len(others):
                        early_units.append(others[i])
                late_units = [
                    (lambda nb=nb: _inproj_q_nb(0, nb)) for nb in range(1, nqb)
                ] + [
                    (lambda nb=nb: _inproj_q_nb(1, nb)) for nb in range(1, nqb)
                ]

                # ---- attention ----
                nkc_last = nkc - 1

                for qb in range(nqb):
                    for hp in range(2):
                        # one bank per head; row 64 of each accumulates Z
                        # (ones-column in v), rows 0-63 accumulate O^T
                        ot_a = ps_ot.tile([65, QB], fp32, name="ot_a", tag="ps_ot")
                        ot_b = ps_ot.tile([65, QB], fp32, name="ot_b", tag="ps_ot")

                        def _emit_av(hp, kc, prob, ot_a=ot_a, ot_b=ot_b):
                            nc.tensor.matmul(
                                ot_a[:, :],
                                v_sb[:, kc, 2 * hp, :],
                                prob[:, 0:QB],
                                start=(kc == 0),
                                stop=(kc == nkc_last),
                            )
                            nc.tensor.matmul(
                                ot_b[:, :],
                                v_sb[:, kc, 2 * hp + 1, :],
                                prob[:, QB : 2 * QB],
                                start=(kc == 0),
                                stop=(kc == nkc_last),
                            )
                        prev = None
                        for kc in range(nkc):
                            sp = ps_qk.tile(
                                [128, 2 * QB], fp32, name="sp", tag="ps_qk"
                            )
                            btab = biasp.tile(
                                [128, 2 * QB], bf16, name="btab", tag="bias"
                            )
                            nc.sync.dma_start(btab[:], biasT_d[hp, kc, qb])
                            nc.tensor.matmul(
                                sp[:, 0:QB],
                                kT_sb[0:64, hp, kc * 128 : (kc + 1) * 128],
                                qT_sb[0:64, hp, qb * QB : (qb + 1) * QB],
                                start=True,
                                stop=True,
                            )
                            nc.tensor.matmul(
                                sp[:, QB : 2 * QB],
                                kT_sb[64:128, hp, kc * 128 : (kc + 1) * 128],
                                qT_sb[64:128, hp, qb * QB : (qb + 1) * QB],
                                start=True,
                                stop=True,
                            )
                            # remaining in-projection drips into qb0's slots
                            if qb == 0:
                                if hp == 0:
                                    for u in early_units[:2]:
                                        u()
                                    del early_units[:2]
                                else:
                                    if early_units:
                                        early_units.pop(0)()
                                    if late_units:
                                        late_units.pop(0)()
                            elif late_units:
                                late_units.pop(0)()
                            # AV for the PREVIOUS chunk goes into the PE queue
                            # here, so it never head-blocks this chunk's QK
                            if prev is not None:
                                _emit_av(hp, prev[0], prev[1])
                            praw = probsp.tile(
                                [128, 2 * QB], bf16, name="praw", tag="praw", bufs=4
                            )
                            nc.scalar.activation(praw[:], sp[:], Exp)
                            # bias applied post-exp: prob = exp(s) * exp(bias),
                            # bf16 DVE 2x mode; padded keys are zeroed in btab
                            prob = probsp.tile(
                                [128, 2 * QB], bf16, name="prob", tag="probs"
                            )
                            nc.vector.tensor_mul(prob[:], praw[:], btab[:])
                            prev = (kc, prob)
                        _emit_av(hp, prev[0], prev[1])
                        # evacuate ot banks promptly (DVE), normalize from SBUF
                        otr = zrecp.tile(
                            [65, 2 * QB], fp32, name="otr", tag="otr", bufs=3
                        )
                        nc.vector.tensor_copy(otr[:, 0:QB], ot_a[:, :])
                        nc.vector.tensor_copy(otr[:, QB : 2 * QB], ot_b[:, :])
                        zrec = zrecp.tile([65, 2 * QB], fp32, name="zrec", tag="zrec")
                        nc.vector.reciprocal(zrec[64:65, 0:QB], otr[64:65, 0:QB])
                        nc.vector.reciprocal(
                            zrec[64:65, QB : 2 * QB], otr[64:65, QB : 2 * QB]
                        )
                        # reuse the just-evacuated ot banks so the big qk
                        # slots stay free for the next head-pair's QK pipeline
                        zb_a = ps_ot.tile([65, QB], fp32, name="zb_a", tag="ps_ot")[
                            0:64, :
                        ]
                        zb_b = ps_ot.tile([65, QB], fp32, name="zb_b", tag="ps_ot")[
                            0:64, :
                        ]
                        nc.tensor.matmul(
                            zb_a[0:64, :],
                            ones_f32[64:65, :],
                            zrec[64:65, 0:QB],
                            start=True,
                            stop=True,
                            tile_position=(64, 0),
                        )
                        nc.tensor.matmul(
                            zb_b[0:64, :],
                            ones_f32[64:65, :],
                            zrec[64:65, QB : 2 * QB],
                            start=True,
                            stop=True,
                            tile_position=(64, 0),
                        )
                        if pack_oproj:
                            # head 2hp   -> otn partitions 0-63 directly
                            # head 2hp+1 -> staging tile, then a DRAM
                            #               round-trip to partitions 64-127
                            #               (K=128 o-proj; direct SBUF->SBUF
                            #               DMA hangs the HW)
                            nc.vector.tensor_mul(
                                otn_sb[0:64, hp, qb * QB : (qb + 1) * QB],
                                otr[0:64, 0:QB],
                                zb_a[:, :],
                            )
                            stg = stagep.tile(
                                [64, QB], bf16, name="stg", tag="stg"
                            )
                            nc.vector.tensor_mul(
                                stg[:], otr[0:64, QB : 2 * QB], zb_b[:, :]
                            )
                            nc.sync.dma_start(stg_d[hp, qb], stg[:])
                            nc.sync.dma_start(
                                otn_sb[64:128, hp, qb * QB : (qb + 1) * QB],
                                stg_d[hp, qb],
                            )
                        else:
                            nc.vector.tensor_mul(
                                otn_sb[:, 2 * hp, qb * QB : (qb + 1) * QB],
                                otr[0:64, 0:QB],
                                zb_a[:, :],
                            )
                            nc.vector.tensor_mul(
                                otn_sb[:, 2 * hp + 1, qb * QB : (qb + 1) * QB],
                                otr[0:64, QB : 2 * QB],
                                zb_b[:, :],
                            )
                    # ---- partial out-projection for this query block ----
                    for lc in range(qb * (QB // 128), (qb + 1) * (QB // 128)):
                        for jb in range(2):
                            pps = ps_out.tile(
                                [128, QB], fp32, name="pps", tag="ps_out"
                            )
                            if pack_oproj:
                                for g in range(2):
                                    nc.tensor.matmul(
                                        pps[:],
                                        otn_sb[:, g, lc * 128 : (lc + 1) * 128],
                                        woT_sb2[:, g, jb * QB : (jb + 1) * QB],
                                        start=(g == 0),
                                        stop=(g == 1),
                                    )
                            else:
                                for hh in range(4):
                                    nc.tensor.matmul(
                                        pps[:],
                                        otn_sb[:, hh, lc * 128 : (lc + 1) * 128],
                                        woT_sb2[:, hh, jb * QB : (jb + 1) * QB],
                                        start=(hh == 0),
                                        stop=(hh == 3),
                                    )
                            osb = outp.tile([128, QB], fp16, name="osb", tag="osb")
                            nc.scalar.copy(osb[:], pps[:])
                            nc.sync.dma_start(
                                out_d[
                                    lc * 128 : (lc + 1) * 128,
                                    jb * QB : (jb + 1) * QB,
                                ],
                                osb[:],
                            )

            if loop_n <= 1:
                _emit()
            elif loop_n % 2 == 0:
                # unroll 2x: iobuf double-buffering lets iteration N+1's
                # in-projection overlap iteration N's attention tail
                with tc.For_i(0, loop_n // 2, 1):
                    _emit()
                    _emit()
            else:
                with tc.For_i(0, loop_n, 1):
                    _emit()

    nc.compile()
    return nc


def _tile_bias(bias4, Lq, Lk):
    """[4, Lq, Lk] -> tiled bf16 [2, nkc, nqb, 128, 1024]:
    [...,:512] = head 2hp (S^T layout: k on partitions), [...,512:] = head 2hp+1."""
    nkc, nqb = Lk // 128, Lq // QB
    bT = bias4.transpose(0, 2, 1).reshape(4, nkc, 128, nqb, QB)
    # [h, kc, p, qb, q] -> [hp, kc, qb, p, ab, q]
    out = np.empty((2, nkc, nqb, 128, 2 * QB), dtype=_BF16)
    for hp in range(2):
        out[hp, :, :, :, 0:QB] = bT[2 * hp].transpose(0, 2, 1, 3).astype(_BF16)
        out[hp, :, :, :, QB:] = bT[2 * hp + 1].transpose(0, 2, 1, 3).astype(_BF16)
    return out


def _shard_inputs(x, key_padding_mask, attn_bias, W_in, b_in, W_out, b_out, Lx=L):
    """Host-side layout prep: slice per core, transpose/cast, gather unpadded
    keys (padded keys have exactly 0 attention mass). Math folded in: q
    weights/bias pre-scaled 1/sqrt(d), bias shipped as exp(attn_bias) over
    kept keys (prob = exp(s)*expb reproduces softmax(s+bias-1e4*pad))."""
    idxs = [np.nonzero(np.asarray(key_padding_mask[b]) == 0)[0] for b in range(B)]
    kept = max(len(i) for i in idxs)
    Lk = max(128, ((kept + 127) // 128) * 128)
    in_maps = []
    W_out_T = np.ascontiguousarray(W_out.T)
    for c in range(8):
        b = c // 4
        h0 = (c % 4) * NHC
        idx = idxs[b]
        nk = len(idx)
        rows_q = slice(h0 * d, (h0 + NHC) * d)
        rows_k = slice(D + h0 * d, D + (h0 + NHC) * d)
        rows_v = slice(2 * D + h0 * d, 2 * D + (h0 + NHC) * d)
        wqk = np.concatenate([W_in[rows_q] / 8.0, W_in[rows_k]], axis=0)  # [512, D]
        wqkT = np.ascontiguousarray(wqk.T).astype(_BF16)
        wvT = np.ascontiguousarray(W_in[rows_v].T).astype(_BF16)
        # out-proj weights packed 2 heads per 128 partitions: row g*128+p
        woT = np.ascontiguousarray(
            W_out_T[rows_q].reshape(2, 128, D).transpose(1, 0, 2)
        ).astype(_BF16)
        woT4 = np.ascontiguousarray(
            W_out_T[rows_q].reshape(4, 64, D).transpose(1, 0, 2)
        ).astype(_BF16)
        bqk_vec = np.concatenate([b_in[rows_q] / 8.0, b_in[rows_k]]).astype(np.float32)
        bqk = np.ascontiguousarray(bqk_vec.reshape(4, 128).T)
        bvr = np.ascontiguousarray(
            np.broadcast_to(b_in[rows_v].astype(np.float32), (128, 256))
        )
        eb = np.zeros((NHC, Lx, Lk), dtype=np.float32)
        eb[:, :, :nk] = np.exp(
            np.asarray(attn_bias[b, h0 : h0 + NHC], dtype=np.float32)[:, :, idx]
        )
        biasT = _tile_bias(eb, Lx, Lk)
        xT = np.ascontiguousarray(x[b].T).astype(_BF16)
        xk = np.zeros((Lk, D), dtype=np.float32)
        xk[:nk] = x[b][idx]
        xkT = np.ascontiguousarray(xk.T).astype(_BF16)
        in_maps.append(
            {
                "xT": xT,
                "xkT": xkT,
                "wqkT": wqkT,
                "wvT": wvT,
                "woT": woT,
                "woT4": woT4,
                "bqk": bqk,
                "bvr": bvr,
                "biasT": biasT,
            }
        )
    return in_maps


def kernel(x, key_padding_mask, attn_bias, W_in, b_in, W_out, b_out):
    from concourse.bass_utils import run_bass_kernel_spmd

    in_maps = _shard_inputs(
        np.asarray(x),
        np.asarray(key_padding_mask),
        np.asarray(attn_bias),
        np.asarray(W_in),
        np.asarray(b_in),
        np.asarray(W_out),
        np.asarray(b_out),
    )
    Lk = in_maps[0]["xkT"].shape[1]
    key = ("nc", Lk, _PACK_OPROJ)
    if key not in _cached:
        _cached[key] = _build_nc(Lk=Lk, pack_oproj=_PACK_OPROJ)
    nc = _cached[key]

    res = run_bass_kernel_spmd(nc, in_maps, core_ids=list(range(8)))
    out = np.empty((B, L, D), dtype=np.float32)
    b_out32 = np.asarray(b_out).astype(np.float32)
    for b in range(B):
        acc = res.results[4 * b]["partial"].astype(np.float32).copy()
        for c in range(4 * b + 1, 4 * b + 4):
            acc += res.results[c]["partial"]
        out[b] = acc + b_out32
    return out
